# revision 8
# baseline (speedup 1.0000x reference)
"""Causal multi-head attention (B=2, S=2048, H=32, D=128) on 8 TRN2 NeuronCores.

Strategy (tensor-parallel over (batch, head) pairs — 64 pairs, 8 per core):

Host side packs per-head inputs into device-friendly layouts:
  qT, kT : [hpc, D, S]  bf16 — Q^T / K^T per head (d on partitions)
  vA     : [hpc, 128, NT*129] bf16 — V tiled [kv-tile, 129] with a ones
           column appended (col 128) so the softmax denominator falls out of
           the PV matmul as an extra output column.
  tri    : [128, 128] bf16 — tri[p, f] = 1 iff p <= f (causal keep-mask for
           diagonal 128x128 blocks in S^T layout).

Device per head:
  S^T[kv, q] tiles = K_tile^T-weights @ Q^T (PE, bf16, fp32 PSUM), packed per
  q-block (512 q columns) into PSUM banks with causal trimming; one big exp
  per PSUM wave on ACT (scale=1/sqrt(D) folded in, no max subtraction —
  scores are O(5) so exp is safe in fp32); causal diagonal fixed by a bf16
  tri-mask multiply on DVE; PV with P^T chunks as the stationary operand so
  the output lands in natural [q, d] layout and the ones column of vA
  accumulates the row sums; normalize with reciprocal + tensor_scalar on DVE.

Upper-triangle blocks are skipped entirely: exp(-1e9) underflows to exactly
0.0 in fp32, so dropping them is bit-equivalent to the reference softmax.
"""

import math

import numpy as np
import ml_dtypes

import concourse.bass as bass
import concourse.mybir as mybir
import concourse.tile as tile
from concourse import bacc
from concourse.tile_rust import add_dep_helper

B, S, H, D = 2, 2048, 32, 128
N_CORES = 8
HPC = (B * H) // N_CORES  # head-pairs per core
VW = D + 1                # V width including the ones column
SCALE = 1.0 / math.sqrt(D)
CHUNK_OFF = (0, 129, 258, 512)  # PV output chunk offsets (chunk 3 in bank 1)
BF16 = mybir.dt.bfloat16
F32 = mybir.dt.float32


def _qblock_layout(qb):
    """Bank-packed S^T layout for q-block qb (512 q cols, kv tiles 0..4qb+3).

    Returns (tiles, nbanks, valid_cols) where tiles is a list of
    (j, col, width, c0): kv-tile j lands at packed column `col`, covering
    local q columns [c0*128, c0*128 + width). Widths are causal-trimmed for
    the 4 diagonal tiles and bin-packed so no matmul crosses a PSUM bank.
    The only slack (256 cols) trails at the very end.
    """
    tiles = []
    bank = 0
    for j in range(qb * 4):
        tiles.append((j, bank * 512, 512, 0))
        bank += 1
    d0 = qb * 4
    tiles.append((d0 + 0, bank * 512, 512, 0))
    bank += 1
    tiles.append((d0 + 1, bank * 512, 384, 1))
    tiles.append((d0 + 3, bank * 512 + 384, 128, 3))
    bank += 1
    tiles.append((d0 + 2, bank * 512, 256, 2))
    bank += 1
    return tiles, bank, (bank - 1) * 512 + 256


def build_module(hpc=HPC, s=S, wave_banks=3):
    nt = s // 128
    qnb = s // 512
    ptw = ((qnb - 1) * 4 + 3) * 512  # widest packed q-block

    nc = bacc.Bacc(trn_type="TRN2")
    qT = nc.dram_tensor("qT", [hpc, D, s], BF16, kind="ExternalInput")
    kT = nc.dram_tensor("kT", [hpc, D, s], BF16, kind="ExternalInput")
    vA = nc.dram_tensor("vA", [hpc, 128, nt * VW], BF16, kind="ExternalInput")
    tri = nc.dram_tensor("tri", [128, 128], BF16, kind="ExternalInput")
    out = nc.dram_tensor("out", [hpc, 128, nt * D], BF16, kind="ExternalOutput")

    exp_fn = mybir.ActivationFunctionType.Exp

    with tile.TileContext(nc) as tc:
        with (
            tc.tile_pool(name="const", bufs=1) as cpool,
            tc.tile_pool(name="io", bufs=2) as iopool,
            tc.tile_pool(name="pt", bufs=4) as ptpool,
            tc.tile_pool(name="ps", bufs=2, space="PSUM") as pspool,
            tc.tile_pool(name="po", bufs=1, space="PSUM") as popool,
            tc.tile_pool(name="nrm", bufs=4) as npool,
            tc.tile_pool(name="un", bufs=2) as unpool,
        ):
            tri_sb = cpool.tile([128, 128], BF16, tag="tri", name="tri_sb")

            # ---- flat wave pipeline across q-blocks and heads ----
            # Per wave: scores matmuls -> exp (ACT) -> diag tri-mask (DVE);
            # PV matmuls trail one wave behind so PE streams wave w+1's
            # scores while ACT runs exp(w) and always has PV work queued.
            # Q-blocks run in descending size so head starts have deep work.
            state = {}     # per-head SBUF tiles
            pending = []   # wave dicts awaiting PV emission (lag queue)
            PV_LAG = 3     # PV trails scores by 3 waves: its exp/tri deps are
                           # guaranteed complete (ps slot WAR), and at q-block
                           # transitions the next QK wave precedes the lagged
                           # PV in PE program order, so ACT is fed first.

            def emit_scores(wv):
                st = wv["st"]
                ps = pspool.tile(
                    [128, wave_banks * 512], F32, tag="ps",
                    name=f"ps{wv['h']}_{wv['qb']}_{wv['wb']}",
                )
                for (j, col, w, c0) in wv["tiles"]:
                    lcol = col - wv["wb"] * 512
                    q0 = wv["qb"] * 512 + c0 * 128
                    nc.tensor.matmul(
                        ps[:, lcol:lcol + w],
                        st["kT"][:, j * 128:(j + 1) * 128],
                        st["qT"][:, q0:q0 + w],
                        start=True, stop=True,
                    )
                ext = min(wv["wn"] * 512, wv["valid"] - wv["wb"] * 512)
                nc.scalar.activation(
                    wv["pt"][:, wv["wb"] * 512: wv["wb"] * 512 + ext],
                    ps[:, 0:ext],
                    exp_fn, scale=SCALE,
                )
                for (j, col, w, c0) in wv["tiles"]:
                    if j >= wv["qb"] * 4:  # diagonal tile: causal mask
                        nc.vector.tensor_mul(
                            wv["pt"][:, col:col + 128],
                            wv["pt"][:, col:col + 128],
                            tri_sb,
                        )

            def emit_pv(wv):
                st = wv["st"]
                if wv["first"]:
                    # PV accumulator for this q-block. The j=0 matmuls of
                    # chunks 0 (bank 0) and 3 (bank 1) carry start=True: the
                    # bank-wide has_written clear makes every other first
                    # write to the bank overwrite-where-clear and later ones
                    # accumulate. Explicit deps pin the start matmul of bank
                    # 0 before its sibling chunks' first writes so Tile
                    # cannot reorder them ahead of the clear.
                    wv["qpo"][0] = popool.tile([128, 1024], F32, tag="po",
                                               name=f"po{wv['h']}_{wv['qb']}")
                po = wv["qpo"][0]
                for (j, col, w, c0) in wv["tiles"]:
                    for c in range(c0, 4):
                        lhsT = wv["pt"][:, col + (c - c0) * 128: col + (c - c0) * 128 + 128]
                        first_write = wv["first"] and j == 0
                        mm = nc.tensor.matmul(
                            po[:, CHUNK_OFF[c]:CHUNK_OFF[c] + VW],
                            lhsT,
                            st["vA"][:, j * VW:(j + 1) * VW],
                            start=first_write and c in (0, 3),
                            stop=False, skip_group_check=True,
                        )
                        if first_write and c == 0:
                            wv["qpo"].append(mm.ins)
                        elif first_write and c in (1, 2):
                            add_dep_helper(mm.ins, wv["qpo"][1], sync=False,
                                           reason="bank0 clear before sibling writes")
                if wv["last"]:
                    h, qb = wv["h"], wv["qb"]
                    if wv["head_last"] and h == HPC - 1:
                        # Kernel tail: normalize straight from PSUM on DVE
                        # (fast cadence, no drain — nothing reuses po after).
                        for c in range(4):
                            qi = qb * 4 + c
                            rc = npool.tile([128, 1], F32, tag="rc",
                                            name=f"rc{h}_{qi}")
                            nc.vector.reciprocal(
                                rc, po[:, CHUNK_OFF[c] + D: CHUNK_OFF[c] + D + 1]
                            )
                            nc.vector.tensor_scalar_mul(
                                st["out"][:, qi * D:(qi + 1) * D],
                                po[:, CHUNK_OFF[c]:CHUNK_OFF[c] + D],
                                rc,
                            )
                    else:
                        # Drain po with one copy (frees both banks), then
                        # normalize on the otherwise-idle Pool engine so DVE
                        # stays free for the tri-masks on the critical path.
                        un = unpool.tile([128, 1024], F32, tag="un",
                                         name=f"un{h}_{qb}")
                        nc.vector.tensor_copy(un[:, 0:CHUNK_OFF[2] + VW],
                                              po[:, 0:CHUNK_OFF[2] + VW])
                        nc.vector.tensor_copy(un[:, 512:512 + VW],
                                              po[:, 512:512 + VW])
                        for c in range(4):
                            qi = qb * 4 + c
                            nc.gpsimd.normalize_recip(
                                st["out"][:, qi * D:(qi + 1) * D],
                                un[:, CHUNK_OFF[c]:CHUNK_OFF[c] + D],
                                un[:, CHUNK_OFF[c] + D: CHUNK_OFF[c] + D + 1],
                            )
                    nc.sync.dma_start(
                        out=out[h][:, qb * 4 * D:(qb + 1) * 4 * D],
                        in_=st["out"][:, qb * 4 * D:(qb + 1) * 4 * D],
                    )

            for h in range(hpc):
                # Tiny first-wave slices first (kT[:, :wave_banks*128] and the
                # last q-block's qT columns) so head 0's first matmuls start
                # as soon as possible; then the bulk.
                w0k = wave_banks * 128
                q0c = (qnb - 1) * 512
                kT_sb = iopool.tile([128, s], BF16, tag="kT", name=f"kT{h}")
                nc.sync.dma_start(out=kT_sb[:, 0:w0k], in_=kT[h][:, 0:w0k])
                qT_sb = iopool.tile([128, s], BF16, tag="qT", name=f"qT{h}")
                nc.sync.dma_start(out=qT_sb[:, q0c:s], in_=qT[h][:, q0c:s])
                if h == 0:
                    nc.sync.dma_start(out=tri_sb, in_=tri[:, :])
                nc.sync.dma_start(out=kT_sb[:, w0k:s], in_=kT[h][:, w0k:s])
                nc.sync.dma_start(out=qT_sb[:, 0:q0c], in_=qT[h][:, 0:q0c])
                vA_sb = iopool.tile([128, nt * VW], BF16, tag="vA", name=f"vA{h}")
                nc.sync.dma_start(out=vA_sb, in_=vA[h])
                out_sb = iopool.tile([128, nt * D], BF16, tag="osb", name=f"osb{h}")
                st = {"kT": kT_sb, "qT": qT_sb, "vA": vA_sb, "out": out_sb}

                for qbi, qb in enumerate(range(qnb - 1, -1, -1)):
                    tiles, nbanks, valid = _qblock_layout(qb)
                    pt_sb = ptpool.tile([128, ptw], BF16, tag="pt", name=f"pt{h}_{qb}")
                    qpo = [None]  # po allocated lazily at first PV of q-block
                    wb = 0
                    qwaves = []
                    while wb < nbanks:
                        wn = min(wave_banks, nbanks - wb)
                        qwaves.append({
                            "h": h, "qb": qb, "wb": wb, "wn": wn,
                            "tiles": [t for t in tiles
                                      if wb * 512 <= t[1] < (wb + wn) * 512],
                            "valid": valid, "pt": pt_sb, "st": st, "qpo": qpo,
                            "first": wb == 0, "last": False,
                            "head_last": False,
                        })
                        wb += wn
                    qwaves[-1]["last"] = True
                    qwaves[-1]["head_last"] = qb == 0
                    for wv in qwaves:
                        emit_scores(wv)
                        pending.append(wv)
                        if len(pending) > PV_LAG:
                            emit_pv(pending.pop(0))
            for wv in pending:
                emit_pv(wv)
    nc.compile()
    return nc


def _pack_inputs(xq, xk, xv, s=S, b=B, h=H):
    """Full [B,S,H,D] fp32 inputs -> per-pair device layouts (bf16)."""
    bf16 = ml_dtypes.bfloat16
    nt = s // 128
    nh = b * h
    # [B,S,H,D] -> [B,H,S,D] -> [nh, S, D]
    q = np.transpose(np.asarray(xq), (0, 2, 1, 3)).reshape(nh, s, D)
    k = np.transpose(np.asarray(xk), (0, 2, 1, 3)).reshape(nh, s, D)
    v = np.transpose(np.asarray(xv), (0, 2, 1, 3)).reshape(nh, s, D)
    qT = np.ascontiguousarray(q.transpose(0, 2, 1)).astype(bf16)  # [nh, D, S]
    kT = np.ascontiguousarray(k.transpose(0, 2, 1)).astype(bf16)
    v4 = v.reshape(nh, nt, 128, D)
    ones = np.ones((nh, nt, 128, 1), np.float32)
    vA = np.concatenate([v4, ones], axis=3)          # [nh, nt, 128, VW]
    vA = np.ascontiguousarray(vA.transpose(0, 2, 1, 3)).reshape(nh, 128, nt * VW)
    vA = vA.astype(bf16)
    tri = np.triu(np.ones((128, 128), np.float32)).astype(bf16)
    return qT, kT, vA, tri


def _unpack_output(outs, s=S, b=B, h=H):
    """Per-core [hpc, 128, NT*D] fp32 -> [B, S, H*D]."""
    nt = s // 128
    o = np.concatenate([np.asarray(x) for x in outs], axis=0)  # [nh, 128, nt*D]
    o = o.reshape(b * h, 128, nt, D).transpose(0, 2, 1, 3)     # [nh, nt, 128, D]
    o = o.reshape(b, h, s, D).transpose(0, 2, 1, 3)            # [B, S, H, D]
    return np.ascontiguousarray(o.reshape(b, s, h * D)).astype(np.float32)


_CACHE = {}


def _get_module():
    if "nc" not in _CACHE:
        _CACHE["nc"] = build_module()
    return _CACHE["nc"]


def make_in_maps(xq, xk, xv):
    qT, kT, vA, tri = _pack_inputs(xq, xk, xv)
    in_maps = []
    for core in range(N_CORES):
        sl = slice(core * HPC, (core + 1) * HPC)
        in_maps.append({
            "qT": np.ascontiguousarray(qT[sl]),
            "kT": np.ascontiguousarray(kT[sl]),
            "vA": np.ascontiguousarray(vA[sl]),
            "tri": tri,
        })
    return in_maps


def kernel(xq, xk, xv, cache_k, cache_v, mask, start_pos):
    assert int(start_pos) == 0, "kernel specialized for start_pos == 0"
    from concourse.bass_utils import run_bass_kernel_spmd

    nc = _get_module()
    in_maps = make_in_maps(xq, xk, xv)
    res = None
    for attempt in range(3):
        try:
            res = run_bass_kernel_spmd(nc, in_maps, core_ids=list(range(N_CORES)))
            break
        except Exception:
            if attempt == 2:
                raise
    outs = [res.results[i]["out"] for i in range(N_CORES)]
    return _unpack_output(outs)



# revision 11
# speedup vs baseline: 1.0157x; 1.0157x over previous
"""Causal multi-head attention (B=2, S=2048, H=32, D=128) on 8 TRN2 NeuronCores.

Strategy (tensor-parallel over (batch, head) pairs — 64 pairs, 8 per core):

Host side packs per-head inputs into device-friendly layouts:
  qT, kT : [hpc, D, S]  bf16 — Q^T / K^T per head (d on partitions)
  vA     : [hpc, 128, NT*129] bf16 — V tiled [kv-tile, 129] with a ones
           column appended (col 128) so the softmax denominator falls out of
           the PV matmul as an extra output column.
  tri    : [128, 128] bf16 — tri[p, f] = 1 iff p <= f (causal keep-mask for
           diagonal 128x128 blocks in S^T layout).

Device per head:
  S^T[kv, q] tiles = K_tile^T-weights @ Q^T (PE, bf16, fp32 PSUM), packed per
  q-block (512 q columns) into PSUM banks with causal trimming; one big exp
  per PSUM wave on ACT (scale=1/sqrt(D) folded in, no max subtraction —
  scores are O(5) so exp is safe in fp32); causal diagonal fixed by a bf16
  tri-mask multiply on DVE; PV with P^T chunks as the stationary operand so
  the output lands in natural [q, d] layout and the ones column of vA
  accumulates the row sums; normalize with reciprocal + tensor_scalar on DVE.

Upper-triangle blocks are skipped entirely: exp(-1e9) underflows to exactly
0.0 in fp32, so dropping them is bit-equivalent to the reference softmax.
"""

import math

import numpy as np
import ml_dtypes

import concourse.bass as bass
import concourse.mybir as mybir
import concourse.tile as tile
from concourse import bacc
from concourse.tile_rust import add_dep_helper

B, S, H, D = 2, 2048, 32, 128
N_CORES = 8
HPC = (B * H) // N_CORES  # head-pairs per core
VW = D + 1                # V width including the ones column
SCALE = 1.0 / math.sqrt(D)
CHUNK_OFF = (0, 129, 258, 512)  # PV output chunk offsets (chunk 3 in bank 1)
BF16 = mybir.dt.bfloat16
F32 = mybir.dt.float32


def _qblock_layout(qb):
    """Bank-packed S^T layout for q-block qb (512 q cols, kv tiles 0..4qb+3).

    Returns (tiles, nbanks, valid_cols) where tiles is a list of
    (j, col, width, c0): kv-tile j lands at packed column `col`, covering
    local q columns [c0*128, c0*128 + width). Widths are causal-trimmed for
    the 4 diagonal tiles and bin-packed so no matmul crosses a PSUM bank.
    The only slack (256 cols) trails at the very end.
    """
    tiles = []
    bank = 0
    for j in range(qb * 4):
        tiles.append((j, bank * 512, 512, 0))
        bank += 1
    d0 = qb * 4
    tiles.append((d0 + 0, bank * 512, 512, 0))
    bank += 1
    tiles.append((d0 + 1, bank * 512, 384, 1))
    tiles.append((d0 + 3, bank * 512 + 384, 128, 3))
    bank += 1
    tiles.append((d0 + 2, bank * 512, 256, 2))
    bank += 1
    return tiles, bank, (bank - 1) * 512 + 256


def build_module(hpc=HPC, s=S, wave_banks=3):
    nt = s // 128
    qnb = s // 512
    ptw = ((qnb - 1) * 4 + 3) * 512  # widest packed q-block

    nc = bacc.Bacc(trn_type="TRN2")
    qT = nc.dram_tensor("qT", [hpc, D, s], BF16, kind="ExternalInput")
    kT = nc.dram_tensor("kT", [hpc, D, s], BF16, kind="ExternalInput")
    vA = nc.dram_tensor("vA", [hpc, 128, nt * VW], BF16, kind="ExternalInput")
    tri = nc.dram_tensor("tri", [128, 128], BF16, kind="ExternalInput")
    out = nc.dram_tensor("out", [hpc, 128, nt * D], BF16, kind="ExternalOutput")

    exp_fn = mybir.ActivationFunctionType.Exp

    with tile.TileContext(nc) as tc:
        with (
            tc.tile_pool(name="const", bufs=1) as cpool,
            tc.tile_pool(name="io", bufs=2) as iopool,
            tc.tile_pool(name="pt", bufs=4) as ptpool,
            tc.tile_pool(name="ps", bufs=2, space="PSUM") as pspool,
            tc.tile_pool(name="po", bufs=1, space="PSUM") as popool,
            tc.tile_pool(name="nrm", bufs=4) as npool,
            tc.tile_pool(name="un", bufs=2) as unpool,
        ):
            tri_sb = cpool.tile([128, 128], BF16, tag="tri", name="tri_sb")

            # ---- flat wave pipeline across q-blocks and heads ----
            # Per wave: scores matmuls -> exp (ACT) -> diag tri-mask (DVE);
            # PV matmuls trail one wave behind so PE streams wave w+1's
            # scores while ACT runs exp(w) and always has PV work queued.
            # Q-blocks run in descending size so head starts have deep work.
            state = {}     # per-head SBUF tiles
            pending = []   # wave dicts awaiting PV emission (lag queue)
            PV_LAG = 2     # PV trails scores by 2 waves: its exp/tri deps are
                           # guaranteed complete (ps slot WAR), so PE never
                           # head-of-line blocks on ACT/DVE.

            def emit_scores(wv):
                st = wv["st"]
                ps = pspool.tile(
                    [128, wave_banks * 512], F32, tag="ps",
                    name=f"ps{wv['h']}_{wv['qb']}_{wv['wb']}",
                )
                for (j, col, w, c0) in wv["tiles"]:
                    lcol = col - wv["wb"] * 512
                    q0 = wv["qb"] * 512 + c0 * 128
                    nc.tensor.matmul(
                        ps[:, lcol:lcol + w],
                        st["kT"][:, j * 128:(j + 1) * 128],
                        st["qT"][:, q0:q0 + w],
                        start=True, stop=True,
                    )
                ext = min(wv["wn"] * 512, wv["valid"] - wv["wb"] * 512)
                nc.scalar.activation(
                    wv["pt"][:, wv["wb"] * 512: wv["wb"] * 512 + ext],
                    ps[:, 0:ext],
                    exp_fn, scale=SCALE,
                )
                for (j, col, w, c0) in wv["tiles"]:
                    if j >= wv["qb"] * 4:  # diagonal tile: causal mask
                        nc.vector.tensor_mul(
                            wv["pt"][:, col:col + 128],
                            wv["pt"][:, col:col + 128],
                            tri_sb,
                        )

            def emit_pv(wv):
                st = wv["st"]
                if wv["first"]:
                    # PV accumulator for this q-block. The j=0 matmuls of
                    # chunks 0 (bank 0) and 3 (bank 1) carry start=True: the
                    # bank-wide has_written clear makes every other first
                    # write to the bank overwrite-where-clear and later ones
                    # accumulate. Explicit deps pin the start matmul of bank
                    # 0 before its sibling chunks' first writes so Tile
                    # cannot reorder them ahead of the clear.
                    wv["qpo"][0] = popool.tile([128, 1024], F32, tag="po",
                                               name=f"po{wv['h']}_{wv['qb']}")
                po = wv["qpo"][0]
                for (j, col, w, c0) in wv["tiles"]:
                    for c in range(c0, 4):
                        lhsT = wv["pt"][:, col + (c - c0) * 128: col + (c - c0) * 128 + 128]
                        first_write = wv["first"] and j == 0
                        mm = nc.tensor.matmul(
                            po[:, CHUNK_OFF[c]:CHUNK_OFF[c] + VW],
                            lhsT,
                            st["vA"][:, j * VW:(j + 1) * VW],
                            start=first_write and c in (0, 3),
                            stop=False, skip_group_check=True,
                        )
                        if first_write and c == 0:
                            wv["qpo"].append(mm.ins)
                        elif first_write and c in (1, 2):
                            add_dep_helper(mm.ins, wv["qpo"][1], sync=False,
                                           reason="bank0 clear before sibling writes")
                if wv["last"]:
                    h, qb = wv["h"], wv["qb"]
                    if wv["head_last"] and h == HPC - 1:
                        # Kernel tail: normalize straight from PSUM on DVE
                        # (fast cadence, no drain — nothing reuses po after).
                        for c in range(4):
                            qi = qb * 4 + c
                            rc = npool.tile([128, 1], F32, tag="rc",
                                            name=f"rc{h}_{qi}")
                            nc.vector.reciprocal(
                                rc, po[:, CHUNK_OFF[c] + D: CHUNK_OFF[c] + D + 1]
                            )
                            nc.vector.tensor_scalar_mul(
                                st["out"][:, qi * D:(qi + 1) * D],
                                po[:, CHUNK_OFF[c]:CHUNK_OFF[c] + D],
                                rc,
                            )
                    else:
                        # Drain po with one copy (frees both banks), then
                        # normalize on the otherwise-idle Pool engine so DVE
                        # stays free for the tri-masks on the critical path.
                        un = unpool.tile([128, 1024], F32, tag="un",
                                         name=f"un{h}_{qb}")
                        nc.vector.tensor_copy(un[:, 0:CHUNK_OFF[2] + VW],
                                              po[:, 0:CHUNK_OFF[2] + VW])
                        nc.vector.tensor_copy(un[:, 512:512 + VW],
                                              po[:, 512:512 + VW])
                        for c in range(4):
                            qi = qb * 4 + c
                            nc.gpsimd.normalize_recip(
                                st["out"][:, qi * D:(qi + 1) * D],
                                un[:, CHUNK_OFF[c]:CHUNK_OFF[c] + D],
                                un[:, CHUNK_OFF[c] + D: CHUNK_OFF[c] + D + 1],
                            )
                    nc.sync.dma_start(
                        out=out[h][:, qb * 4 * D:(qb + 1) * 4 * D],
                        in_=st["out"][:, qb * 4 * D:(qb + 1) * 4 * D],
                    )

            for h in range(hpc):
                # Tiny first-wave slices first (q-block 0 needs only the
                # first 512 kv cols of kT and first 512 q cols of qT) so
                # head 0's first matmuls start as soon as possible.
                w0k = 512
                kT_sb = iopool.tile([128, s], BF16, tag="kT", name=f"kT{h}")
                nc.sync.dma_start(out=kT_sb[:, 0:w0k], in_=kT[h][:, 0:w0k])
                qT_sb = iopool.tile([128, s], BF16, tag="qT", name=f"qT{h}")
                nc.sync.dma_start(out=qT_sb[:, 0:w0k], in_=qT[h][:, 0:w0k])
                if h == 0:
                    nc.sync.dma_start(out=tri_sb, in_=tri[:, :])
                nc.sync.dma_start(out=kT_sb[:, w0k:s], in_=kT[h][:, w0k:s])
                nc.sync.dma_start(out=qT_sb[:, w0k:s], in_=qT[h][:, w0k:s])
                vA_sb = iopool.tile([128, nt * VW], BF16, tag="vA", name=f"vA{h}")
                nc.sync.dma_start(out=vA_sb, in_=vA[h])
                out_sb = iopool.tile([128, nt * D], BF16, tag="osb", name=f"osb{h}")
                st = {"kT": kT_sb, "qT": qT_sb, "vA": vA_sb, "out": out_sb}

                # Ascending q-block order: the small q-blocks (1 and 3 waves)
                # run at head start, where ACT still has the previous head's
                # big-wave backlog queued; the big uniform q-blocks land at
                # the head tail where the pipeline would otherwise drain.
                for qbi, qb in enumerate(range(qnb)):
                    tiles, nbanks, valid = _qblock_layout(qb)
                    pt_sb = ptpool.tile([128, ptw], BF16, tag="pt", name=f"pt{h}_{qb}")
                    qpo = [None]  # po allocated lazily at first PV of q-block
                    wb = 0
                    qwaves = []
                    while wb < nbanks:
                        wn = min(wave_banks, nbanks - wb)
                        qwaves.append({
                            "h": h, "qb": qb, "wb": wb, "wn": wn,
                            "tiles": [t for t in tiles
                                      if wb * 512 <= t[1] < (wb + wn) * 512],
                            "valid": valid, "pt": pt_sb, "st": st, "qpo": qpo,
                            "first": wb == 0, "last": False,
                            "head_last": False,
                        })
                        wb += wn
                    qwaves[-1]["last"] = True
                    qwaves[-1]["head_last"] = qb == qnb - 1
                    for wv in qwaves:
                        emit_scores(wv)
                        pending.append(wv)
                        if len(pending) > PV_LAG:
                            emit_pv(pending.pop(0))
            for wv in pending:
                emit_pv(wv)
    nc.compile()
    return nc


def _pack_inputs(xq, xk, xv, s=S, b=B, h=H):
    """Full [B,S,H,D] fp32 inputs -> per-pair device layouts (bf16)."""
    bf16 = ml_dtypes.bfloat16
    nt = s // 128
    nh = b * h
    # [B,S,H,D] -> [B,H,S,D] -> [nh, S, D]
    q = np.transpose(np.asarray(xq), (0, 2, 1, 3)).reshape(nh, s, D)
    k = np.transpose(np.asarray(xk), (0, 2, 1, 3)).reshape(nh, s, D)
    v = np.transpose(np.asarray(xv), (0, 2, 1, 3)).reshape(nh, s, D)
    qT = np.ascontiguousarray(q.transpose(0, 2, 1)).astype(bf16)  # [nh, D, S]
    kT = np.ascontiguousarray(k.transpose(0, 2, 1)).astype(bf16)
    v4 = v.reshape(nh, nt, 128, D)
    ones = np.ones((nh, nt, 128, 1), np.float32)
    vA = np.concatenate([v4, ones], axis=3)          # [nh, nt, 128, VW]
    vA = np.ascontiguousarray(vA.transpose(0, 2, 1, 3)).reshape(nh, 128, nt * VW)
    vA = vA.astype(bf16)
    tri = np.triu(np.ones((128, 128), np.float32)).astype(bf16)
    return qT, kT, vA, tri


def _unpack_output(outs, s=S, b=B, h=H):
    """Per-core [hpc, 128, NT*D] fp32 -> [B, S, H*D]."""
    nt = s // 128
    o = np.concatenate([np.asarray(x) for x in outs], axis=0)  # [nh, 128, nt*D]
    o = o.reshape(b * h, 128, nt, D).transpose(0, 2, 1, 3)     # [nh, nt, 128, D]
    o = o.reshape(b, h, s, D).transpose(0, 2, 1, 3)            # [B, S, H, D]
    return np.ascontiguousarray(o.reshape(b, s, h * D)).astype(np.float32)


_CACHE = {}


def _get_module():
    if "nc" not in _CACHE:
        _CACHE["nc"] = build_module()
    return _CACHE["nc"]


def make_in_maps(xq, xk, xv):
    qT, kT, vA, tri = _pack_inputs(xq, xk, xv)
    in_maps = []
    for core in range(N_CORES):
        sl = slice(core * HPC, (core + 1) * HPC)
        in_maps.append({
            "qT": np.ascontiguousarray(qT[sl]),
            "kT": np.ascontiguousarray(kT[sl]),
            "vA": np.ascontiguousarray(vA[sl]),
            "tri": tri,
        })
    return in_maps


def kernel(xq, xk, xv, cache_k, cache_v, mask, start_pos):
    assert int(start_pos) == 0, "kernel specialized for start_pos == 0"
    from concourse.bass_utils import run_bass_kernel_spmd

    nc = _get_module()
    in_maps = make_in_maps(xq, xk, xv)
    res = None
    for attempt in range(3):
        try:
            res = run_bass_kernel_spmd(nc, in_maps, core_ids=list(range(N_CORES)))
            break
        except Exception:
            if attempt == 2:
                raise
    outs = [res.results[i]["out"] for i in range(N_CORES)]
    return _unpack_output(outs)



# revision 14
# speedup vs baseline: 1.0208x; 1.0051x over previous
"""Causal multi-head attention (B=2, S=2048, H=32, D=128) on 8 TRN2 NeuronCores.

Strategy (tensor-parallel over (batch, head) pairs — 64 pairs, 8 per core):

Host side packs per-head inputs into device-friendly layouts:
  qT, kT : [hpc, D, S]  bf16 — Q^T / K^T per head (d on partitions)
  vA     : [hpc, 128, NT*129] bf16 — V tiled [kv-tile, 129] with a ones
           column appended (col 128) so the softmax denominator falls out of
           the PV matmul as an extra output column.
  tri    : [128, 128] bf16 — tri[p, f] = 1 iff p <= f (causal keep-mask for
           diagonal 128x128 blocks in S^T layout).

Device per head:
  S^T[kv, q] tiles = K_tile^T-weights @ Q^T (PE, bf16, fp32 PSUM), packed per
  q-block (512 q columns) into PSUM banks with causal trimming; one big exp
  per PSUM wave on ACT (scale=1/sqrt(D) folded in, no max subtraction —
  scores are O(5) so exp is safe in fp32); causal diagonal fixed by a bf16
  tri-mask multiply on DVE; PV with P^T chunks as the stationary operand so
  the output lands in natural [q, d] layout and the ones column of vA
  accumulates the row sums; normalize with reciprocal + tensor_scalar on DVE.

Upper-triangle blocks are skipped entirely: exp(-1e9) underflows to exactly
0.0 in fp32, so dropping them is bit-equivalent to the reference softmax.
"""

import math

import numpy as np
import ml_dtypes

import concourse.bass as bass
import concourse.mybir as mybir
import concourse.tile as tile
from concourse import bacc
from concourse.tile_rust import add_dep_helper

B, S, H, D = 2, 2048, 32, 128
N_CORES = 8
HPC = (B * H) // N_CORES  # head-pairs per core
VW = D + 1                # V width including the ones column
SCALE = 1.0 / math.sqrt(D)
CHUNK_OFF = (0, 129, 258, 512)  # PV output chunk offsets (chunk 3 in bank 1)
BF16 = mybir.dt.bfloat16
F32 = mybir.dt.float32


def _qblock_layout(qb):
    """Bank-packed S^T layout for q-block qb (512 q cols, kv tiles 0..4qb+3).

    Returns (tiles, nbanks, valid_cols) where tiles is a list of
    (j, col, width, c0): kv-tile j lands at packed column `col`, covering
    local q columns [c0*128, c0*128 + width). Widths are causal-trimmed for
    the 4 diagonal tiles and bin-packed so no matmul crosses a PSUM bank.
    The only slack (256 cols) trails at the very end.
    """
    tiles = []
    bank = 0
    for j in range(qb * 4):
        tiles.append((j, bank * 512, 512, 0))
        bank += 1
    d0 = qb * 4
    tiles.append((d0 + 0, bank * 512, 512, 0))
    bank += 1
    tiles.append((d0 + 1, bank * 512, 384, 1))
    tiles.append((d0 + 3, bank * 512 + 384, 128, 3))
    bank += 1
    tiles.append((d0 + 2, bank * 512, 256, 2))
    bank += 1
    return tiles, bank, (bank - 1) * 512 + 256


def build_module(hpc=HPC, s=S, wave_banks=3):
    nt = s // 128
    qnb = s // 512
    ptw = ((qnb - 1) * 4 + 3) * 512  # widest packed q-block

    nc = bacc.Bacc(trn_type="TRN2")
    qT = nc.dram_tensor("qT", [hpc, D, s], BF16, kind="ExternalInput")
    kT = nc.dram_tensor("kT", [hpc, D, s], BF16, kind="ExternalInput")
    vA = nc.dram_tensor("vA", [hpc, 128, nt * VW], BF16, kind="ExternalInput")
    tri = nc.dram_tensor("tri", [128, 128], BF16, kind="ExternalInput")
    out = nc.dram_tensor("out", [hpc, 128, nt * D], BF16, kind="ExternalOutput")

    exp_fn = mybir.ActivationFunctionType.Exp

    with tile.TileContext(nc) as tc:
        with (
            tc.tile_pool(name="const", bufs=1) as cpool,
            tc.tile_pool(name="io", bufs=2) as iopool,
            tc.tile_pool(name="pt", bufs=4) as ptpool,
            tc.tile_pool(name="ps", bufs=2, space="PSUM") as pspool,
            tc.tile_pool(name="po", bufs=1, space="PSUM") as popool,
            tc.tile_pool(name="nrm", bufs=4) as npool,
            tc.tile_pool(name="un", bufs=2) as unpool,
        ):
            tri_sb = cpool.tile([128, 128], BF16, tag="tri", name="tri_sb")

            # PE warm-up: the PE ramps 0.65 -> 2.4 GHz over ~3us of
            # continuous execution. Dummy matmuls on zeros keep it busy
            # (and fully ramped) while the first input DMAs land, so the
            # first real QK wave runs at full clock. They share ps slot
            # rotation; real writes overwrite with start=True.
            zpad = cpool.tile([128, 512], BF16, tag="zpad", name="zpad")
            nc.gpsimd.memset(zpad, 0.0)
            for wu in range(10):
                ps_w = pspool.tile([128, wave_banks * 512], F32, tag="ps",
                                   name=f"warm{wu}")
                for b in range(wave_banks):
                    nc.tensor.matmul(
                        ps_w[:, b * 512:(b + 1) * 512],
                        zpad[:, 0:128], zpad,
                        start=True, stop=True,
                    )

            # ---- flat wave pipeline across q-blocks and heads ----
            # Per wave: scores matmuls -> exp (ACT) -> diag tri-mask (DVE);
            # PV matmuls trail one wave behind so PE streams wave w+1's
            # scores while ACT runs exp(w) and always has PV work queued.
            # Q-blocks run in descending size so head starts have deep work.
            state = {}     # per-head SBUF tiles
            pending = []   # wave dicts awaiting PV emission (lag queue)
            PV_LAG = 2     # PV trails scores by 2 waves: its exp/tri deps are
                           # guaranteed complete (ps slot WAR), so PE never
                           # head-of-line blocks on ACT/DVE.

            def emit_scores(wv):
                st = wv["st"]
                ps = pspool.tile(
                    [128, wave_banks * 512], F32, tag="ps",
                    name=f"ps{wv['h']}_{wv['qb']}_{wv['wb']}",
                )
                # High priority: the wave ACT consumes next must never queue
                # behind lagged-PV filler in the PE stream.
                with tc.high_priority(offset=150):
                    for (j, col, w, c0) in wv["tiles"]:
                        lcol = col - wv["wb"] * 512
                        q0 = wv["qb"] * 512 + c0 * 128
                        nc.tensor.matmul(
                            ps[:, lcol:lcol + w],
                            st["kT"][:, j * 128:(j + 1) * 128],
                            st["qT"][:, q0:q0 + w],
                            start=True, stop=True,
                        )
                ext = min(wv["wn"] * 512, wv["valid"] - wv["wb"] * 512)
                nc.scalar.activation(
                    wv["pt"][:, wv["wb"] * 512: wv["wb"] * 512 + ext],
                    ps[:, 0:ext],
                    exp_fn, scale=SCALE,
                )
                for (j, col, w, c0) in wv["tiles"]:
                    if j >= wv["qb"] * 4:  # diagonal tile: causal mask
                        nc.vector.tensor_mul(
                            wv["pt"][:, col:col + 128],
                            wv["pt"][:, col:col + 128],
                            tri_sb,
                        )

            def emit_pv(wv):
                st = wv["st"]
                if wv["first"]:
                    # PV accumulator for this q-block. The j=0 matmuls of
                    # chunks 0 (bank 0) and 3 (bank 1) carry start=True: the
                    # bank-wide has_written clear makes every other first
                    # write to the bank overwrite-where-clear and later ones
                    # accumulate. Explicit deps pin the start matmul of bank
                    # 0 before its sibling chunks' first writes so Tile
                    # cannot reorder them ahead of the clear.
                    wv["qpo"][0] = popool.tile([128, 1024], F32, tag="po",
                                               name=f"po{wv['h']}_{wv['qb']}")
                po = wv["qpo"][0]
                for (j, col, w, c0) in wv["tiles"]:
                    for c in range(c0, 4):
                        lhsT = wv["pt"][:, col + (c - c0) * 128: col + (c - c0) * 128 + 128]
                        first_write = wv["first"] and j == 0
                        mm = nc.tensor.matmul(
                            po[:, CHUNK_OFF[c]:CHUNK_OFF[c] + VW],
                            lhsT,
                            st["vA"][:, j * VW:(j + 1) * VW],
                            start=first_write and c in (0, 3),
                            stop=False, skip_group_check=True,
                        )
                        if first_write and c == 0:
                            wv["qpo"].append(mm.ins)
                        elif first_write and c in (1, 2):
                            add_dep_helper(mm.ins, wv["qpo"][1], sync=False,
                                           reason="bank0 clear before sibling writes")
                if wv["last"]:
                    h, qb = wv["h"], wv["qb"]
                    if wv["head_last"] and h == HPC - 1:
                        # Kernel tail: normalize straight from PSUM on DVE
                        # (fast cadence, no drain — nothing reuses po after).
                        for c in range(4):
                            qi = qb * 4 + c
                            rc = npool.tile([128, 1], F32, tag="rc",
                                            name=f"rc{h}_{qi}")
                            nc.vector.reciprocal(
                                rc, po[:, CHUNK_OFF[c] + D: CHUNK_OFF[c] + D + 1]
                            )
                            nc.vector.tensor_scalar_mul(
                                st["out"][:, qi * D:(qi + 1) * D],
                                po[:, CHUNK_OFF[c]:CHUNK_OFF[c] + D],
                                rc,
                            )
                    else:
                        # Drain po with one copy (frees both banks), then
                        # normalize on the otherwise-idle Pool engine so DVE
                        # stays free for the tri-masks on the critical path.
                        un = unpool.tile([128, 1024], F32, tag="un",
                                         name=f"un{h}_{qb}")
                        nc.vector.tensor_copy(un[:, 0:CHUNK_OFF[2] + VW],
                                              po[:, 0:CHUNK_OFF[2] + VW])
                        nc.vector.tensor_copy(un[:, 512:512 + VW],
                                              po[:, 512:512 + VW])
                        for c in range(4):
                            qi = qb * 4 + c
                            nc.gpsimd.normalize_recip(
                                st["out"][:, qi * D:(qi + 1) * D],
                                un[:, CHUNK_OFF[c]:CHUNK_OFF[c] + D],
                                un[:, CHUNK_OFF[c] + D: CHUNK_OFF[c] + D + 1],
                            )
                    nc.sync.dma_start(
                        out=out[h][:, qb * 4 * D:(qb + 1) * 4 * D],
                        in_=st["out"][:, qb * 4 * D:(qb + 1) * 4 * D],
                    )

            for h in range(hpc):
                # Tiny first-wave slices first (q-block 0 needs only the
                # first 512 kv cols of kT and first 512 q cols of qT) so
                # head 0's first matmuls start as soon as possible.
                w0k = 512
                kT_sb = iopool.tile([128, s], BF16, tag="kT", name=f"kT{h}")
                nc.sync.dma_start(out=kT_sb[:, 0:w0k], in_=kT[h][:, 0:w0k])
                qT_sb = iopool.tile([128, s], BF16, tag="qT", name=f"qT{h}")
                nc.sync.dma_start(out=qT_sb[:, 0:w0k], in_=qT[h][:, 0:w0k])
                if h == 0:
                    nc.sync.dma_start(out=tri_sb, in_=tri[:, :])
                nc.sync.dma_start(out=kT_sb[:, w0k:s], in_=kT[h][:, w0k:s])
                nc.sync.dma_start(out=qT_sb[:, w0k:s], in_=qT[h][:, w0k:s])
                vA_sb = iopool.tile([128, nt * VW], BF16, tag="vA", name=f"vA{h}")
                nc.sync.dma_start(out=vA_sb, in_=vA[h])
                out_sb = iopool.tile([128, nt * D], BF16, tag="osb", name=f"osb{h}")
                st = {"kT": kT_sb, "qT": qT_sb, "vA": vA_sb, "out": out_sb}

                # Ascending q-block order: the small q-blocks (1 and 3 waves)
                # run at head start, where ACT still has the previous head's
                # big-wave backlog queued; the big uniform q-blocks land at
                # the head tail where the pipeline would otherwise drain.
                for qbi, qb in enumerate(range(qnb)):
                    tiles, nbanks, valid = _qblock_layout(qb)
                    pt_sb = ptpool.tile([128, ptw], BF16, tag="pt", name=f"pt{h}_{qb}")
                    qpo = [None]  # po allocated lazily at first PV of q-block
                    wb = 0
                    qwaves = []
                    while wb < nbanks:
                        wn = min(wave_banks, nbanks - wb)
                        qwaves.append({
                            "h": h, "qb": qb, "wb": wb, "wn": wn,
                            "tiles": [t for t in tiles
                                      if wb * 512 <= t[1] < (wb + wn) * 512],
                            "valid": valid, "pt": pt_sb, "st": st, "qpo": qpo,
                            "first": wb == 0, "last": False,
                            "head_last": False,
                        })
                        wb += wn
                    qwaves[-1]["last"] = True
                    qwaves[-1]["head_last"] = qb == qnb - 1
                    for wv in qwaves:
                        emit_scores(wv)
                        pending.append(wv)
                        if len(pending) > PV_LAG:
                            emit_pv(pending.pop(0))
            for wv in pending:
                emit_pv(wv)
    nc.compile()
    return nc


def _pack_inputs(xq, xk, xv, s=S, b=B, h=H):
    """Full [B,S,H,D] fp32 inputs -> per-pair device layouts (bf16)."""
    bf16 = ml_dtypes.bfloat16
    nt = s // 128
    nh = b * h
    # [B,S,H,D] -> [B,H,S,D] -> [nh, S, D]
    q = np.transpose(np.asarray(xq), (0, 2, 1, 3)).reshape(nh, s, D)
    k = np.transpose(np.asarray(xk), (0, 2, 1, 3)).reshape(nh, s, D)
    v = np.transpose(np.asarray(xv), (0, 2, 1, 3)).reshape(nh, s, D)
    qT = np.ascontiguousarray(q.transpose(0, 2, 1)).astype(bf16)  # [nh, D, S]
    kT = np.ascontiguousarray(k.transpose(0, 2, 1)).astype(bf16)
    v4 = v.reshape(nh, nt, 128, D)
    ones = np.ones((nh, nt, 128, 1), np.float32)
    vA = np.concatenate([v4, ones], axis=3)          # [nh, nt, 128, VW]
    vA = np.ascontiguousarray(vA.transpose(0, 2, 1, 3)).reshape(nh, 128, nt * VW)
    vA = vA.astype(bf16)
    tri = np.triu(np.ones((128, 128), np.float32)).astype(bf16)
    return qT, kT, vA, tri


def _unpack_output(outs, s=S, b=B, h=H):
    """Per-core [hpc, 128, NT*D] fp32 -> [B, S, H*D]."""
    nt = s // 128
    o = np.concatenate([np.asarray(x) for x in outs], axis=0)  # [nh, 128, nt*D]
    o = o.reshape(b * h, 128, nt, D).transpose(0, 2, 1, 3)     # [nh, nt, 128, D]
    o = o.reshape(b, h, s, D).transpose(0, 2, 1, 3)            # [B, S, H, D]
    return np.ascontiguousarray(o.reshape(b, s, h * D)).astype(np.float32)


_CACHE = {}


def _get_module():
    if "nc" not in _CACHE:
        _CACHE["nc"] = build_module()
    return _CACHE["nc"]


def make_in_maps(xq, xk, xv):
    qT, kT, vA, tri = _pack_inputs(xq, xk, xv)
    in_maps = []
    for core in range(N_CORES):
        sl = slice(core * HPC, (core + 1) * HPC)
        in_maps.append({
            "qT": np.ascontiguousarray(qT[sl]),
            "kT": np.ascontiguousarray(kT[sl]),
            "vA": np.ascontiguousarray(vA[sl]),
            "tri": tri,
        })
    return in_maps


def kernel(xq, xk, xv, cache_k, cache_v, mask, start_pos):
    assert int(start_pos) == 0, "kernel specialized for start_pos == 0"
    from concourse.bass_utils import run_bass_kernel_spmd

    nc = _get_module()
    in_maps = make_in_maps(xq, xk, xv)
    res = None
    for attempt in range(3):
        try:
            res = run_bass_kernel_spmd(nc, in_maps, core_ids=list(range(N_CORES)))
            break
        except Exception:
            if attempt == 2:
                raise
    outs = [res.results[i]["out"] for i in range(N_CORES)]
    return _unpack_output(outs)



# revision 16
# speedup vs baseline: 1.0265x; 1.0055x over previous
"""Causal multi-head attention (B=2, S=2048, H=32, D=128) on 8 TRN2 NeuronCores.

Strategy (tensor-parallel over (batch, head) pairs — 64 pairs, 8 per core):

Host side packs per-head inputs into device-friendly layouts:
  qT, kT : [hpc, D, S]  bf16 — Q^T / K^T per head (d on partitions)
  vA     : [hpc, 128, NT*129] bf16 — V tiled [kv-tile, 129] with a ones
           column appended (col 128) so the softmax denominator falls out of
           the PV matmul as an extra output column.
  tri    : [128, 128] bf16 — tri[p, f] = 1 iff p <= f (causal keep-mask for
           diagonal 128x128 blocks in S^T layout).

Device per head — dense bank-packed wave pipeline:
  All causal S^T pieces of a head (kv-tile x q-block, causally trimmed,
  split at 512-col PSUM bank boundaries) are packed gapless into a 34-bank
  stream, cut into 12 waves (1x512 + 11x1536).  Per wave: QK matmuls into a
  double-buffered 3-bank PSUM tile (PE, bf16), one exp over the whole wave
  (ACT, scale folded in, no max subtraction — scores are O(5) so fp32 exp is
  safe) into a per-head pt tile in SBUF, diag squares fixed by a bf16
  tri-mask multiply (DVE).  PV trails two waves behind with P^T chunks as
  the stationary operand, so output lands in [q, d] layout and the vA ones
  column accumulates row sums; per q-block the accumulator is drained (DVE)
  and normalized on the otherwise-idle Pool engine, then DMA'd out as bf16.

Uniform waves keep the PE->ACT 2-slot PSUM rotation cadence jitter-free:
ACT (the bottleneck at ~134us/core) sees near-identical 1536-col work items
back to back.  Upper-triangle blocks are skipped entirely: exp(-1e9)
underflows to exactly 0.0 in fp32, so dropping them is bit-equivalent to
the reference softmax.
"""

import math

import numpy as np
import ml_dtypes

import concourse.bass as bass
import concourse.mybir as mybir
import concourse.tile as tile
from concourse import bacc
from concourse.tile_rust import add_dep_helper

B, S, H, D = 2, 2048, 32, 128
N_CORES = 8
HPC = (B * H) // N_CORES  # head-pairs per core
VW = D + 1                # V width including the ones column
SCALE = 1.0 / math.sqrt(D)
CHUNK_OFF = (0, 129, 258, 512)  # PV output chunk offsets (chunk 3 in bank 1)
BF16 = mybir.dt.bfloat16
F32 = mybir.dt.float32


def _head_layout(s=S):
    """Dense bank-packed causal S^T piece list for one head.

    Pieces are (j, qb, c0, nch, pos, square): kv-tile j of q-block qb,
    chunks c0..c0+nch-1 (128-col q-chunks within the q-block), packed at
    global column pos.  `square` is True when the piece's first chunk is
    the diagonal square needing the tri mask.  Pieces never cross 512-col
    PSUM bank boundaries (split on the fly); widths/positions are all
    multiples of 128 so chunks stay 128-aligned.
    """
    pieces = []
    pos = 0
    qnb = s // 512
    for qb in range(qnb):
        for j in range(4 * qb + 4):
            c0 = max(0, j - 4 * qb)
            nch = 4 - c0
            first = True
            while nch > 0:
                room = (512 - pos % 512) // 128
                n = min(nch, room)
                pieces.append((j, qb, c0, n, pos, first and c0 == j - 4 * qb))
                pos += n * 128
                c0 += n
                nch -= n
                first = False
    return pieces, pos


def build_module(hpc=HPC, s=S, wave_banks=3):
    nt = s // 128
    qnb = s // 512
    pieces, head_cols = _head_layout(s)
    # wave cut points: one short 512 wave first, then uniform 1536 waves
    cuts = [0, 512]
    while cuts[-1] < head_cols:
        cuts.append(min(cuts[-1] + wave_banks * 512, head_cols))
    waves_per_head = len(cuts) - 1

    nc = bacc.Bacc(trn_type="TRN2")
    qT = nc.dram_tensor("qT", [hpc, D, s], BF16, kind="ExternalInput")
    kT = nc.dram_tensor("kT", [hpc, D, s], BF16, kind="ExternalInput")
    vA = nc.dram_tensor("vA", [hpc, 128, nt * VW], BF16, kind="ExternalInput")
    tri = nc.dram_tensor("tri", [128, 128], BF16, kind="ExternalInput")
    out = nc.dram_tensor("out", [hpc, 128, nt * D], BF16, kind="ExternalOutput")

    exp_fn = mybir.ActivationFunctionType.Exp

    with tile.TileContext(nc) as tc:
        with (
            tc.tile_pool(name="const", bufs=1) as cpool,
            tc.tile_pool(name="io", bufs=2) as iopool,
            tc.tile_pool(name="pt", bufs=2) as ptpool,
            tc.tile_pool(name="ps", bufs=2, space="PSUM") as pspool,
            tc.tile_pool(name="po", bufs=1, space="PSUM") as popool,
            tc.tile_pool(name="nrm", bufs=4) as npool,
            tc.tile_pool(name="un", bufs=2) as unpool,
        ):
            tri_sb = cpool.tile([128, 128], BF16, tag="tri", name="tri_sb")

            # PE warm-up: the PE ramps 0.65 -> 2.4 GHz over ~3us of
            # continuous execution.  Dummy matmuls on zeros keep it busy
            # (and ramping) while the first input DMAs land.  They share
            # the ps slot rotation; real writes overwrite with start=True.
            zpad = cpool.tile([128, 512], BF16, tag="zpad", name="zpad")
            nc.gpsimd.memset(zpad, 0.0)
            for wu in range(2):
                ps_w = pspool.tile([128, wave_banks * 512], F32, tag="ps",
                                   name=f"warm{wu}")
                for b in range(wave_banks):
                    nc.tensor.matmul(
                        ps_w[:, b * 512:(b + 1) * 512],
                        zpad[:, 0:128], zpad,
                        start=True, stop=True,
                    )

            # ---- flat uniform-wave pipeline across heads ----
            pending = []   # wave dicts awaiting PV emission (lag queue)
            PV_LAG = 2     # PV trails scores by 2 waves: its exp/tri deps
                           # are complete by then (ps slot WAR), so PE never
                           # head-of-line blocks on ACT/DVE.
            qb_last_piece = {}
            for idx, p in enumerate(pieces):
                qb_last_piece[p[1]] = idx

            def emit_scores(wv):
                st = wv["st"]
                w0, w1 = wv["lo"], wv["hi"]
                ps = pspool.tile(
                    [128, wave_banks * 512], F32, tag="ps",
                    name=f"ps{wv['h']}_{wv['wi']}",
                )
                # High priority: the wave ACT consumes next must never queue
                # behind lagged-PV filler in the PE stream.
                with tc.high_priority(offset=150):
                    for (j, qb, c0, nch, pos, sq) in wv["pieces"]:
                        w = nch * 128
                        q0 = qb * 512 + c0 * 128
                        nc.tensor.matmul(
                            ps[:, pos - w0:pos - w0 + w],
                            st["kT"][:, j * 128:(j + 1) * 128],
                            st["qT"][:, q0:q0 + w],
                            start=True, stop=True,
                        )
                nc.scalar.activation(
                    wv["pt"][:, w0:w1], ps[:, 0:w1 - w0], exp_fn, scale=SCALE,
                )
                for (j, qb, c0, nch, pos, sq) in wv["pieces"]:
                    if sq:  # diagonal square: causal mask
                        nc.vector.tensor_mul(
                            wv["pt"][:, pos:pos + 128],
                            wv["pt"][:, pos:pos + 128],
                            tri_sb,
                        )

            def emit_pv(wv):
                st = wv["st"]
                h = wv["h"]
                qs = wv["qstate"]
                for pi, (j, qb, c0, nch, pos, sq) in zip(
                        wv["pidx"], wv["pieces"]):
                    if qb not in qs:
                        # PV accumulator for this q-block.  start=True on the
                        # first write to each bank clears it bank-wide
                        # (has_written), so every other first write to the
                        # bank lands overwrite-where-clear and later ones
                        # accumulate.  Explicit deps pin the bank-0 clear
                        # before its sibling chunks' first writes.
                        qs[qb] = {
                            "po": popool.tile([128, 1024], F32, tag="po",
                                              name=f"po{h}_{qb}"),
                            "banks": set(), "clear": None,
                        }
                    q = qs[qb]
                    po = q["po"]
                    for c in range(c0, c0 + nch):
                        bank = 1 if c == 3 else 0
                        first_bank = bank not in q["banks"]
                        q["banks"].add(bank)
                        mm = nc.tensor.matmul(
                            po[:, CHUNK_OFF[c]:CHUNK_OFF[c] + VW],
                            wv["pt"][:, pos + (c - c0) * 128:
                                     pos + (c - c0) * 128 + 128],
                            st["vA"][:, j * VW:(j + 1) * VW],
                            start=first_bank,
                            stop=False, skip_group_check=True,
                        )
                        if bank == 0:
                            if first_bank:
                                q["clear"] = mm.ins
                            elif c not in q.setdefault("written", set()):
                                add_dep_helper(
                                    mm.ins, q["clear"], sync=False,
                                    reason="bank0 clear before sibling writes")
                        q.setdefault("written", set()).add(c)
                    if pi == qb_last_piece[qb]:
                        emit_norm(wv, qb, q["po"])

            def emit_norm(wv, qb, po):
                st = wv["st"]
                h = wv["h"]
                if h == hpc - 1 and qb == qnb - 1:
                    # Kernel tail: normalize straight from PSUM on DVE
                    # (fast cadence, no drain — nothing reuses po after).
                    for c in range(4):
                        qi = qb * 4 + c
                        rc = npool.tile([128, 1], F32, tag="rc",
                                        name=f"rc{h}_{qi}")
                        nc.vector.reciprocal(
                            rc, po[:, CHUNK_OFF[c] + D: CHUNK_OFF[c] + D + 1]
                        )
                        nc.vector.tensor_scalar_mul(
                            st["out"][:, qi * D:(qi + 1) * D],
                            po[:, CHUNK_OFF[c]:CHUNK_OFF[c] + D],
                            rc,
                        )
                else:
                    # Drain po with one copy (frees both banks), then
                    # normalize on the otherwise-idle Pool engine so DVE
                    # stays free for the tri-masks on the critical path.
                    un = unpool.tile([128, 1024], F32, tag="un",
                                     name=f"un{h}_{qb}")
                    nc.vector.tensor_copy(un[:, 0:CHUNK_OFF[2] + VW],
                                          po[:, 0:CHUNK_OFF[2] + VW])
                    nc.vector.tensor_copy(un[:, 512:512 + VW],
                                          po[:, 512:512 + VW])
                    for c in range(4):
                        qi = qb * 4 + c
                        nc.gpsimd.normalize_recip(
                            st["out"][:, qi * D:(qi + 1) * D],
                            un[:, CHUNK_OFF[c]:CHUNK_OFF[c] + D],
                            un[:, CHUNK_OFF[c] + D: CHUNK_OFF[c] + D + 1],
                        )
                nc.sync.dma_start(
                    out=out[h][:, qb * 4 * D:(qb + 1) * 4 * D],
                    in_=st["out"][:, qb * 4 * D:(qb + 1) * 4 * D],
                )

            for h in range(hpc):
                # Tiny first-wave slices first (wave 0 needs only kT[:, :128]
                # and qT[:, :512]) so head 0's first matmuls start as soon as
                # possible; then the bulk.
                kT_sb = iopool.tile([128, s], BF16, tag="kT", name=f"kT{h}")
                nc.sync.dma_start(out=kT_sb[:, 0:512], in_=kT[h][:, 0:512])
                qT_sb = iopool.tile([128, s], BF16, tag="qT", name=f"qT{h}")
                nc.sync.dma_start(out=qT_sb[:, 0:512], in_=qT[h][:, 0:512])
                if h == 0:
                    nc.sync.dma_start(out=tri_sb, in_=tri[:, :])
                nc.sync.dma_start(out=kT_sb[:, 512:s], in_=kT[h][:, 512:s])
                nc.sync.dma_start(out=qT_sb[:, 512:s], in_=qT[h][:, 512:s])
                vA_sb = iopool.tile([128, nt * VW], BF16, tag="vA", name=f"vA{h}")
                nc.sync.dma_start(out=vA_sb, in_=vA[h])
                out_sb = iopool.tile([128, nt * D], BF16, tag="osb", name=f"osb{h}")
                st = {"kT": kT_sb, "qT": qT_sb, "vA": vA_sb, "out": out_sb}

                pt_sb = ptpool.tile([128, head_cols], BF16, tag="pt",
                                    name=f"pt{h}")
                qstate = {}
                for wi in range(waves_per_head):
                    lo, hi = cuts[wi], cuts[wi + 1]
                    wsel = [(i, p) for i, p in enumerate(pieces)
                            if lo <= p[4] < hi]
                    wv = {
                        "h": h, "wi": wi, "lo": lo, "hi": hi,
                        "pieces": [p for _, p in wsel],
                        "pidx": [i for i, _ in wsel],
                        "pt": pt_sb, "st": st, "qstate": qstate,
                    }
                    emit_scores(wv)
                    pending.append(wv)
                    if len(pending) > PV_LAG:
                        emit_pv(pending.pop(0))
            for wv in pending:
                emit_pv(wv)
    nc.compile()
    return nc


def _pack_inputs(xq, xk, xv, s=S, b=B, h=H):
    """Full [B,S,H,D] fp32 inputs -> per-pair device layouts (bf16)."""
    bf16 = ml_dtypes.bfloat16
    nt = s // 128
    nh = b * h
    # [B,S,H,D] -> [B,H,S,D] -> [nh, S, D]
    q = np.transpose(np.asarray(xq), (0, 2, 1, 3)).reshape(nh, s, D)
    k = np.transpose(np.asarray(xk), (0, 2, 1, 3)).reshape(nh, s, D)
    v = np.transpose(np.asarray(xv), (0, 2, 1, 3)).reshape(nh, s, D)
    qT = np.ascontiguousarray(q.transpose(0, 2, 1)).astype(bf16)  # [nh, D, S]
    kT = np.ascontiguousarray(k.transpose(0, 2, 1)).astype(bf16)
    v4 = v.reshape(nh, nt, 128, D)
    ones = np.ones((nh, nt, 128, 1), np.float32)
    vA = np.concatenate([v4, ones], axis=3)          # [nh, nt, 128, VW]
    vA = np.ascontiguousarray(vA.transpose(0, 2, 1, 3)).reshape(nh, 128, nt * VW)
    vA = vA.astype(bf16)
    tri = np.triu(np.ones((128, 128), np.float32)).astype(bf16)
    return qT, kT, vA, tri


def _unpack_output(outs, s=S, b=B, h=H):
    """Per-core [hpc, 128, NT*D] bf16 -> [B, S, H*D] fp32."""
    nt = s // 128
    o = np.concatenate([np.asarray(x) for x in outs], axis=0)  # [nh, 128, nt*D]
    o = o.reshape(b * h, 128, nt, D).transpose(0, 2, 1, 3)     # [nh, nt, 128, D]
    o = o.reshape(b, h, s, D).transpose(0, 2, 1, 3)            # [B, S, H, D]
    return np.ascontiguousarray(o.reshape(b, s, h * D)).astype(np.float32)


_CACHE = {}


def _get_module():
    if "nc" not in _CACHE:
        _CACHE["nc"] = build_module()
    return _CACHE["nc"]


def make_in_maps(xq, xk, xv):
    qT, kT, vA, tri = _pack_inputs(xq, xk, xv)
    in_maps = []
    for core in range(N_CORES):
        sl = slice(core * HPC, (core + 1) * HPC)
        in_maps.append({
            "qT": np.ascontiguousarray(qT[sl]),
            "kT": np.ascontiguousarray(kT[sl]),
            "vA": np.ascontiguousarray(vA[sl]),
            "tri": tri,
        })
    return in_maps


def kernel(xq, xk, xv, cache_k, cache_v, mask, start_pos):
    assert int(start_pos) == 0, "kernel specialized for start_pos == 0"
    from concourse.bass_utils import run_bass_kernel_spmd

    nc = _get_module()
    in_maps = make_in_maps(xq, xk, xv)
    res = None
    for attempt in range(3):
        try:
            res = run_bass_kernel_spmd(nc, in_maps, core_ids=list(range(N_CORES)))
            break
        except Exception:
            if attempt == 2:
                raise
    outs = [res.results[i]["out"] for i in range(N_CORES)]
    return _unpack_output(outs)


# revision 18
# speedup vs baseline: 1.0373x; 1.0106x over previous
"""Causal multi-head attention (B=2, S=2048, H=32, D=128) on 8 TRN2 NeuronCores.

Strategy (tensor-parallel over (batch, head) pairs — 64 pairs, 8 per core):

Host side packs per-head inputs into device-friendly layouts:
  qT, kT : [hpc, D, S]  bf16 — Q^T / K^T per head (d on partitions)
  vA     : [hpc, 128, NT*129] bf16 — V tiled [kv-tile, 129] with a ones
           column appended (col 128) so the softmax denominator falls out of
           the PV matmul as an extra output column.
  tri    : [128, 128] bf16 — tri[p, f] = 1 iff p <= f (causal keep-mask for
           diagonal 128x128 blocks in S^T layout).

Device — one gapless bank-packed wave pipeline over the whole core:
  All causal S^T pieces (kv-tile x q-block, causally trimmed, split at
  512-col PSUM bank boundaries) of all 8 heads form a dense 272-bank
  stream, cut into uniform 3-bank (1536-col) waves; the only short waves
  (512 cols) are the very first and last, where the pipeline is filling or
  draining anyway.  Waves may straddle a head boundary (the exp is then
  split per head segment).  Per wave: QK matmuls into a double-buffered
  3-bank PSUM tile (PE, bf16), one exp per head-segment (ACT, scale folded
  in; no max subtraction — scores are O(5) so fp32 exp is safe) into a
  per-head pt tile in SBUF, diagonal squares fixed by a bf16 tri-mask
  multiply (DVE).  PV trails two waves behind with P^T chunks as the
  stationary operand, so output lands in [q, d] layout and the vA ones
  column accumulates row sums; per q-block the accumulator is drained
  (DVE) and normalized on the otherwise-idle Pool engine, then DMA'd out
  as bf16.

Uniform waves keep the PE->ACT 2-slot PSUM rotation cadence jitter-free:
ACT (the bottleneck at ~134us/core) sees near-identical 1536-col work
items back to back for the entire kernel.  Upper-triangle blocks are
skipped entirely: exp(-1e9) underflows to exactly 0.0 in fp32, so
dropping them is bit-equivalent to the reference softmax.
"""

import math

import numpy as np
import ml_dtypes

import concourse.bass as bass
import concourse.mybir as mybir
import concourse.tile as tile
from concourse import bacc
from concourse.tile_rust import add_dep_helper

B, S, H, D = 2, 2048, 32, 128
N_CORES = 8
HPC = (B * H) // N_CORES  # head-pairs per core
VW = D + 1                # V width including the ones column
SCALE = 1.0 / math.sqrt(D)
CHUNK_OFF = (0, 129, 258, 512)  # PV output chunk offsets (chunk 3 in bank 1)
BF16 = mybir.dt.bfloat16
F32 = mybir.dt.float32


def _head_layout(s=S):
    """Dense bank-packed causal S^T piece list for one head.

    Pieces are (j, qb, c0, nch, pos, square): kv-tile j of q-block qb,
    chunks c0..c0+nch-1 (128-col q-chunks within the q-block), packed at
    column pos.  `square` marks the piece whose first chunk is the diagonal
    square needing the tri mask.  Pieces never cross 512-col PSUM bank
    boundaries (split on the fly); widths/positions are all multiples of
    128 so chunks stay 128-aligned.
    """
    pieces = []
    pos = 0
    qnb = s // 512
    for qb in range(qnb):
        for j in range(4 * qb + 4):
            c0 = max(0, j - 4 * qb)
            nch = 4 - c0
            first = True
            while nch > 0:
                room = (512 - pos % 512) // 128
                n = min(nch, room)
                pieces.append((j, qb, c0, n, pos, first and c0 == j - 4 * qb))
                pos += n * 128
                c0 += n
                nch -= n
                first = False
    return pieces, pos


def build_module(hpc=HPC, s=S, wave_banks=3):
    nt = s // 128
    qnb = s // 512
    hpieces, head_cols = _head_layout(s)
    # Global piece stream: (h, j, qb, c0, nch, pos-within-head, square)
    gpieces = [(h, *p) for h in range(hpc) for p in hpieces]
    total_cols = hpc * head_cols
    # Wave cut points over global columns: short first wave (quick kernel
    # start), uniform 1536 interior waves, short remainder at the end.
    cuts = [0, 512]
    while cuts[-1] < total_cols:
        cuts.append(min(cuts[-1] + wave_banks * 512, total_cols))

    nc = bacc.Bacc(trn_type="TRN2")
    qT = nc.dram_tensor("qT", [hpc, D, s], BF16, kind="ExternalInput")
    kT = nc.dram_tensor("kT", [hpc, D, s], BF16, kind="ExternalInput")
    vA = nc.dram_tensor("vA", [hpc, 128, nt * VW], BF16, kind="ExternalInput")
    tri = nc.dram_tensor("tri", [128, 128], BF16, kind="ExternalInput")
    out = nc.dram_tensor("out", [hpc, 128, nt * D], BF16, kind="ExternalOutput")

    exp_fn = mybir.ActivationFunctionType.Exp

    last_piece = {}
    for idx, (h, j, qb, c0, nch, pos, sq) in enumerate(gpieces):
        last_piece[(h, qb)] = idx

    with tile.TileContext(nc) as tc:
        with (
            tc.tile_pool(name="const", bufs=1) as cpool,
            tc.tile_pool(name="io", bufs=2) as iopool,
            tc.tile_pool(name="pt", bufs=2) as ptpool,
            tc.tile_pool(name="ps", bufs=2, space="PSUM") as pspool,
            tc.tile_pool(name="po", bufs=1, space="PSUM") as popool,
            tc.tile_pool(name="nrm", bufs=4) as npool,
            tc.tile_pool(name="un", bufs=2) as unpool,
        ):
            tri_sb = cpool.tile([128, 128], BF16, tag="tri", name="tri_sb")
            head_st = {}
            qstate = {}
            pending = []   # wave dicts awaiting PV emission (lag queue)
            PV_LAG = 2     # PV trails scores by 2 waves: its exp/tri deps
                           # are complete by then (ps slot WAR), so PE never
                           # head-of-line blocks on ACT/DVE.

            def emit_head_dma(h):
                # Tiny first-wave slices first so head 0's first matmuls
                # start as soon as possible; 512-col slices keep later
                # waves' needs ahead of the bulk.
                kT_sb = iopool.tile([128, s], BF16, tag="kT", name=f"kT{h}")
                qT_sb = iopool.tile([128, s], BF16, tag="qT", name=f"qT{h}")
                vA_sb = iopool.tile([128, nt * VW], BF16, tag="vA",
                                    name=f"vA{h}")
                if h == 0:
                    nc.sync.dma_start(out=kT_sb[:, 0:512], in_=kT[h][:, 0:512])
                    nc.sync.dma_start(out=qT_sb[:, 0:512], in_=qT[h][:, 0:512])
                    nc.sync.dma_start(out=tri_sb, in_=tri[:, :])
                    nc.sync.dma_start(out=kT_sb[:, 512:1024],
                                      in_=kT[h][:, 512:1024])
                    nc.sync.dma_start(out=qT_sb[:, 512:1024],
                                      in_=qT[h][:, 512:1024])
                    nc.sync.dma_start(out=vA_sb[:, 0:8 * VW],
                                      in_=vA[h][:, 0:8 * VW])
                    nc.sync.dma_start(out=kT_sb[:, 1024:s],
                                      in_=kT[h][:, 1024:s])
                    nc.sync.dma_start(out=qT_sb[:, 1024:s],
                                      in_=qT[h][:, 1024:s])
                    nc.sync.dma_start(out=vA_sb[:, 8 * VW:],
                                      in_=vA[h][:, 8 * VW:])
                else:
                    nc.sync.dma_start(out=kT_sb, in_=kT[h])
                    nc.sync.dma_start(out=qT_sb, in_=qT[h])
                    nc.sync.dma_start(out=vA_sb, in_=vA[h])
                out_sb = iopool.tile([128, nt * D], BF16, tag="osb",
                                     name=f"osb{h}")
                pt_sb = ptpool.tile([128, head_cols], BF16, tag="pt",
                                    name=f"pt{h}")
                head_st[h] = {"kT": kT_sb, "qT": qT_sb, "vA": vA_sb,
                              "out": out_sb, "pt": pt_sb}

            def emit_scores(wv):
                w0, w1 = wv["lo"], wv["hi"]
                ps = pspool.tile(
                    [128, wave_banks * 512], F32, tag="ps",
                    name=f"ps{wv['wi']}",
                )
                # High priority: the wave ACT consumes next must never
                # queue behind lagged-PV filler in the PE stream.
                with tc.high_priority(offset=150):
                    for (h, j, qb, c0, nch, pos, sq) in wv["pieces"]:
                        st = head_st[h]
                        w = nch * 128
                        q0 = qb * 512 + c0 * 128
                        g = h * head_cols + pos
                        nc.tensor.matmul(
                            ps[:, g - w0:g - w0 + w],
                            st["kT"][:, j * 128:(j + 1) * 128],
                            st["qT"][:, q0:q0 + w],
                            start=True, stop=True,
                        )
                # one exp per head segment of the wave
                for h, lo, hi in wv["segs"]:
                    nc.scalar.activation(
                        head_st[h]["pt"][:, lo:hi],
                        ps[:, h * head_cols + lo - w0:
                           h * head_cols + hi - w0],
                        exp_fn, scale=SCALE,
                    )
                for (h, j, qb, c0, nch, pos, sq) in wv["pieces"]:
                    if sq:  # diagonal square: causal mask
                        pt = head_st[h]["pt"]
                        nc.vector.tensor_mul(
                            pt[:, pos:pos + 128], pt[:, pos:pos + 128], tri_sb,
                        )

            def emit_pv(wv):
                for pi, (h, j, qb, c0, nch, pos, sq) in zip(
                        wv["pidx"], wv["pieces"]):
                    st = head_st[h]
                    key = (h, qb)
                    if key not in qstate:
                        # PV accumulator for this q-block.  start=True on
                        # the first write to each bank clears it bank-wide
                        # (has_written), so every other first write to the
                        # bank lands overwrite-where-clear and later ones
                        # accumulate.  Explicit deps pin the bank-0 clear
                        # before its sibling chunks' first writes.
                        qstate[key] = {
                            "po": popool.tile([128, 1024], F32, tag="po",
                                              name=f"po{h}_{qb}"),
                            "banks": set(), "clear": None, "written": set(),
                        }
                    q = qstate[key]
                    po = q["po"]
                    for c in range(c0, c0 + nch):
                        bank = 1 if c == 3 else 0
                        first_bank = bank not in q["banks"]
                        q["banks"].add(bank)
                        mm = nc.tensor.matmul(
                            po[:, CHUNK_OFF[c]:CHUNK_OFF[c] + VW],
                            st["pt"][:, pos + (c - c0) * 128:
                                     pos + (c - c0) * 128 + 128],
                            st["vA"][:, j * VW:(j + 1) * VW],
                            start=first_bank,
                            stop=False, skip_group_check=True,
                        )
                        if bank == 0:
                            if first_bank:
                                q["clear"] = mm.ins
                            elif c not in q["written"]:
                                add_dep_helper(
                                    mm.ins, q["clear"], sync=False,
                                    reason="bank0 clear before sibling writes")
                        q["written"].add(c)
                    if pi == last_piece[(h, qb)]:
                        emit_norm(h, qb, q["po"])

            def emit_norm(h, qb, po):
                st = head_st[h]
                if h == hpc - 1 and qb == qnb - 1:
                    # Kernel tail: normalize straight from PSUM on DVE
                    # (fast cadence, no drain — nothing reuses po after).
                    for c in range(4):
                        qi = qb * 4 + c
                        rc = npool.tile([128, 1], F32, tag="rc",
                                        name=f"rc{h}_{qi}")
                        nc.vector.reciprocal(
                            rc, po[:, CHUNK_OFF[c] + D: CHUNK_OFF[c] + D + 1]
                        )
                        nc.vector.tensor_scalar_mul(
                            st["out"][:, qi * D:(qi + 1) * D],
                            po[:, CHUNK_OFF[c]:CHUNK_OFF[c] + D],
                            rc,
                        )
                else:
                    # Drain po with one copy (frees both banks), then
                    # normalize on the otherwise-idle Pool engine so DVE
                    # stays free for the tri-masks on the critical path.
                    un = unpool.tile([128, 1024], F32, tag="un",
                                     name=f"un{h}_{qb}")
                    nc.vector.tensor_copy(un[:, 0:CHUNK_OFF[2] + VW],
                                          po[:, 0:CHUNK_OFF[2] + VW])
                    nc.vector.tensor_copy(un[:, 512:512 + VW],
                                          po[:, 512:512 + VW])
                    for c in range(4):
                        qi = qb * 4 + c
                        nc.gpsimd.normalize_recip(
                            st["out"][:, qi * D:(qi + 1) * D],
                            un[:, CHUNK_OFF[c]:CHUNK_OFF[c] + D],
                            un[:, CHUNK_OFF[c] + D: CHUNK_OFF[c] + D + 1],
                        )
                nc.sync.dma_start(
                    out=out[h][:, qb * 4 * D:(qb + 1) * 4 * D],
                    in_=st["out"][:, qb * 4 * D:(qb + 1) * 4 * D],
                )

            # ---- walk the global wave stream ----
            pi = 0
            for wi in range(len(cuts) - 1):
                lo, hi = cuts[wi], cuts[wi + 1]
                wsel = []
                while pi < len(gpieces):
                    h, j, qb, c0, nch, pos, sq = gpieces[pi]
                    g = h * head_cols + pos
                    if g >= hi:
                        break
                    wsel.append((pi, gpieces[pi]))
                    pi += 1
                # per-head exp segments [lo, hi) in head-local cols
                segs = []
                for _, (h, j, qb, c0, nch, pos, sq) in wsel:
                    g0, g1 = h * head_cols + pos, h * head_cols + pos + nch * 128
                    if segs and segs[-1][0] == h:
                        segs[-1][2] = pos + nch * 128
                    else:
                        segs.append([h, pos, pos + nch * 128])
                wv = {
                    "wi": wi, "lo": lo, "hi": hi,
                    "pieces": [p for _, p in wsel],
                    "pidx": [i for i, _ in wsel],
                    "segs": segs,
                }
                for _, (h, *_rest) in wsel:
                    if h not in head_st:
                        emit_head_dma(h)
                    # prefetch the next head a full head period ahead
                    if h + 1 < hpc and h + 1 not in head_st:
                        emit_head_dma(h + 1)
                emit_scores(wv)
                pending.append(wv)
                if len(pending) > PV_LAG:
                    emit_pv(pending.pop(0))
            for wv in pending:
                emit_pv(wv)
    nc.compile()
    return nc


def _pack_inputs(xq, xk, xv, s=S, b=B, h=H):
    """Full [B,S,H,D] fp32 inputs -> per-pair device layouts (bf16)."""
    bf16 = ml_dtypes.bfloat16
    nt = s // 128
    nh = b * h
    # [B,S,H,D] -> [B,H,S,D] -> [nh, S, D]
    q = np.transpose(np.asarray(xq), (0, 2, 1, 3)).reshape(nh, s, D)
    k = np.transpose(np.asarray(xk), (0, 2, 1, 3)).reshape(nh, s, D)
    v = np.transpose(np.asarray(xv), (0, 2, 1, 3)).reshape(nh, s, D)
    qT = np.ascontiguousarray(q.transpose(0, 2, 1)).astype(bf16)  # [nh, D, S]
    kT = np.ascontiguousarray(k.transpose(0, 2, 1)).astype(bf16)
    v4 = v.reshape(nh, nt, 128, D)
    ones = np.ones((nh, nt, 128, 1), np.float32)
    vA = np.concatenate([v4, ones], axis=3)          # [nh, nt, 128, VW]
    vA = np.ascontiguousarray(vA.transpose(0, 2, 1, 3)).reshape(nh, 128, nt * VW)
    vA = vA.astype(bf16)
    tri = np.triu(np.ones((128, 128), np.float32)).astype(bf16)
    return qT, kT, vA, tri


def _unpack_output(outs, s=S, b=B, h=H):
    """Per-core [hpc, 128, NT*D] bf16 -> [B, S, H*D] fp32."""
    nt = s // 128
    o = np.concatenate([np.asarray(x) for x in outs], axis=0)  # [nh, 128, nt*D]
    o = o.reshape(b * h, 128, nt, D).transpose(0, 2, 1, 3)     # [nh, nt, 128, D]
    o = o.reshape(b, h, s, D).transpose(0, 2, 1, 3)            # [B, S, H, D]
    return np.ascontiguousarray(o.reshape(b, s, h * D)).astype(np.float32)


_CACHE = {}


def _get_module():
    if "nc" not in _CACHE:
        _CACHE["nc"] = build_module()
    return _CACHE["nc"]


def make_in_maps(xq, xk, xv):
    qT, kT, vA, tri = _pack_inputs(xq, xk, xv)
    in_maps = []
    for core in range(N_CORES):
        sl = slice(core * HPC, (core + 1) * HPC)
        in_maps.append({
            "qT": np.ascontiguousarray(qT[sl]),
            "kT": np.ascontiguousarray(kT[sl]),
            "vA": np.ascontiguousarray(vA[sl]),
            "tri": tri,
        })
    return in_maps


def kernel(xq, xk, xv, cache_k, cache_v, mask, start_pos):
    assert int(start_pos) == 0, "kernel specialized for start_pos == 0"
    from concourse.bass_utils import run_bass_kernel_spmd

    nc = _get_module()
    in_maps = make_in_maps(xq, xk, xv)
    res = None
    for attempt in range(3):
        try:
            res = run_bass_kernel_spmd(nc, in_maps, core_ids=list(range(N_CORES)))
            break
        except Exception:
            if attempt == 2:
                raise
    outs = [res.results[i]["out"] for i in range(N_CORES)]
    return _unpack_output(outs)
